# revision 1
# baseline (speedup 1.0000x reference)
"""DepthIoULoss kernel for Trainium2 (Bass/Tile), data-parallel over 8 cores.

Math (per row, S segments; v[-1] treated as 0): with M = min(p, t) and
X = max(p, t) elementwise:
    inter_j = relu(M_j - X_{j-1});  union_j = X_j - M_{j-1};  iou = inter/union
Valid prefix: j <= stop_idx, where stop_idx = first index with t == 1.0.
row_iou = sum_valid iou_j / (stop_idx + 1);  loss = 1 - mean_rows(row_iou).

Device plan per [128, 2048] row-tile (only ops this walrus build accepts):
  ACT    tq  = t * K                      (K = 1e9, Copy w/ scale)
  DVE    cmx = exclusive-cummax(tq)       (tensor_tensor_scan max, shifted AP)
  DVE    M   = min(p, t);  X = max(p, t)  (min/max are DVE-only here;
                                           [128, S+1] tiles, zero column 0)
  GPSIMD i0  = M[:,1:] - X[:,:-1]
  GPSIMD u0  = X[:,1:] - M[:,:-1]
  DVE    u'  = max(cmx - 0.95K, u0), accum -> ia    (invalid lanes -> V=0.05K;
                                                     ia = n_invalid*V + O(1e3))
  ACT    lnu = Ln(u');  r = Exp(-lnu) = 1/u'        (invalid -> 2e-8)
  DVE    junk= max(i0,0) * r, accum -> rowsum       (relu fused here)
Epilogue on [128, 8]: num_seg = S - round(ia / V) (2^23 magic rounding),
row_iou = rowsum / num_seg -> DMA out. Host: loss = 1 - sum(row_iou) / B.

Manual software pipelining: engine queues run in EMISSION order, so the
um pass (which waits on GPSIMD's u0) is emitted one tile late and the
Ln/Exp/final passes two tiles late. This hides the Pool and ACT latency
behind the next tile's DVE work: sim went 140 us -> 104 us per core.

The masked lanes contribute |inter|/V <= 2048 * 2e-8 ~ 4e-5 absolute to a
rowsum of O(1..30): negligible. num_seg recovery is exact (error << 0.5).
"""

import numpy as np

B, S = 8192, 2048
NCORES = 8
ROWS_PER_CORE = B // NCORES  # 1024
TILES = ROWS_PER_CORE // 128  # 8

K_SCALE = np.float32(1.0e9)  # ACT Ln accurate to ~1e16; keep u' moderate
C_THRESH = np.float32(0.95) * K_SCALE
V_INVALID = float(np.float32(K_SCALE - np.float32(C_THRESH)))  # invalid-lane u'
MAGIC = 8388608.0  # 2**23: float add/sub rounds to nearest integer

C_SPLIT = 1728  # DVE computes max on cols [0,C_SPLIT); Pool derives the rest
                # as (p+t) - min  (1-ulp difference, numerically validated)

UM_SKEW = 1  # um pass trails stage A by one tile
B_SKEW = 2  # ln/exp/final trail stage A by two tiles

_NC_CACHE = None

_RANGE_CLEAR_OPCODE = 176  # EVENT_SEMAPHORE_RANGE_CLEAR


def _legalize_waits(nc, maxw=1):
    """Make the Tile-generated module compatible with this walrus build.

    1. Drop tail EVENT_SEMAPHORE_RANGE_CLEAR InstISA ops (NRT re-initializes
       semaphore state per execution; this walrus rejects the encoding).
    2. Split instructions carrying more than `maxw` sync waits: excess waits
       move to carrier EventSemaphore nops inserted just before, same engine.
    """
    import concourse.mybir as mybir

    uid = [0]
    for fn in nc.m.functions:
        for blk in fn.blocks:
            lst = blk.instructions
            k = 0
            while k < len(lst):
                inst = lst[k]
                if (
                    type(inst).__name__ == "InstISA"
                    and getattr(inst, "isa_opcode", None) == _RANGE_CLEAR_OPCODE
                ):
                    si = inst.sync_info
                    if si is not None and (si.on_wait or si.on_update):
                        carrier = mybir.InstEventSemaphore(name=f"RCW-{uid[0]}")
                        uid[0] += 1
                        carrier.engine = inst.engine
                        carrier.sync_info = si
                        lst[k] = carrier
                        k += 1
                    else:
                        del lst[k]
                    continue
                si = inst.sync_info
                if si is not None and si.on_wait and len(si.on_wait) > maxw:
                    waits = list(si.on_wait)
                    extra, keep = waits[:-maxw], waits[-maxw:]
                    pos = k
                    for j in range(0, len(extra), maxw):
                        carrier = mybir.InstEventSemaphore(name=f"EVW-{uid[0]}")
                        uid[0] += 1
                        carrier.engine = inst.engine
                        carrier.sync_info = mybir.SyncInfo(
                            on_wait=extra[j : j + maxw], on_update=[]
                        )
                        lst.insert(pos, carrier)
                        pos += 1
                        k += 1
                    inst.sync_info = mybir.SyncInfo(
                        on_wait=keep, on_update=list(si.on_update)
                    )
                k += 1
    return nc


def _build_nc():
    import concourse.bass as bass
    import concourse.mybir as mybir
    from concourse.tile import TileContext

    f32 = mybir.dt.float32
    alu = mybir.AluOpType
    act = mybir.ActivationFunctionType

    nc = bass.Bass()
    p_d = nc.dram_tensor("predictions", [ROWS_PER_CORE, S], f32, kind="ExternalInput")
    t_d = nc.dram_tensor("targets", [ROWS_PER_CORE, S], f32, kind="ExternalInput")
    o_d = nc.dram_tensor("row_iou", [128, TILES], f32, kind="ExternalOutput")

    with TileContext(nc) as tc:
        with (
            tc.tile_pool(name="io", bufs=2) as iop,
            tc.tile_pool(name="geom", bufs=2) as gp,
            tc.tile_pool(name="cmxp", bufs=3) as cp,
            tc.tile_pool(name="i0p", bufs=2) as i0p,
            tc.tile_pool(name="u0p", bufs=2) as u0p,
            tc.tile_pool(name="ump", bufs=3) as ump,
            tc.tile_pool(name="uch", bufs=2) as up,
            tc.tile_pool(name="sp", bufs=2) as spp,
            tc.tile_pool(name="smp", bufs=1) as smp,
        ):
            acc_sb = smp.tile([128, TILES], f32, tag="acc")
            rs_sb = smp.tile([128, TILES], f32, tag="rs")
            carr = smp.tile([128, TILES], f32, tag="carr")
            nc.vector.memset(carr[:], float(S) + MAGIC)
            w_sb = smp.tile([128, TILES], f32, tag="w")
            w2_sb = smp.tile([128, TILES], f32, tag="w2")

            st_a = {}
            st_u = {}

            def stage_a(i):
                rows = slice(i * 128, (i + 1) * 128)
                p = iop.tile([128, S], f32, tag="p")
                t = iop.tile([128, S], f32, tag="t")
                tq = gp.tile([128, S], f32, tag="tq")
                cmx = cp.tile([128, S], f32, tag="cmx")
                M = gp.tile([128, S + 1], f32, tag="M")
                X = gp.tile([128, S + 1], f32, tag="X")
                sv = spp.tile([128, S - C_SPLIT], f32, tag="s")
                nc.scalar.memzero(cmx[:, 0:1])
                nc.scalar.memzero(M[:, 0:1])
                nc.gpsimd.memset(X[:, 0:1], 0.0)
                # tile 0 is processed in column quarters so DVE ramps up while
                # the rest of the data is still in flight (chained scan).
                nparts = 4 if i == 0 else 1
                Hc = S // nparts
                for k in range(nparts):
                    a, b = k * Hc, (k + 1) * Hc
                    nc.sync.dma_start(out=t[:, a:b], in_=t_d[rows, a:b])
                    nc.sync.dma_start(out=p[:, a:b], in_=p_d[rows, a:b])
                    nc.scalar.activation(
                        out=tq[:, a:b], in_=t[:, a:b], func=act.Copy,
                        scale=float(K_SCALE),
                    )
                    # min first: Pool's derived-max part waits on M
                    nc.vector.tensor_tensor(
                        out=M[:, a + 1 : b + 1], in0=p[:, a:b], in1=t[:, a:b],
                        op=alu.min,
                    )
                    last = k == nparts - 1
                    nc.vector.tensor_tensor_scan(
                        out=cmx[:, a + 1 : (b if last else b + 1)],
                        data0=tq[:, a : (b - 1 if last else b)],
                        data1=tq[:, a : (b - 1 if last else b)],
                        initial=(0.0 if k == 0 else cmx[:, a : a + 1]),
                        op0=alu.max,
                        op1=alu.bypass,
                    )
                    hi = min(b, C_SPLIT)
                    if hi > a:
                        nc.vector.tensor_tensor(
                            out=X[:, a + 1 : hi + 1], in0=p[:, a:hi],
                            in1=t[:, a:hi], op=alu.max,
                        )
                # Pool derives the remaining max columns: X = (p+t) - M
                nc.gpsimd.tensor_tensor(
                    out=sv[:], in0=p[:, C_SPLIT:S], in1=t[:, C_SPLIT:S],
                    op=alu.add,
                )
                nc.gpsimd.tensor_tensor(
                    out=X[:, C_SPLIT + 1 : S + 1], in0=sv[:],
                    in1=M[:, C_SPLIT + 1 : S + 1], op=alu.subtract,
                )
                i0 = i0p.tile([128, S], f32, tag="i0")
                nc.gpsimd.tensor_tensor(
                    out=i0[:], in0=M[:, 1 : S + 1], in1=X[:, 0:S], op=alu.subtract
                )
                u0 = u0p.tile([128, S], f32, tag="u0")
                nc.gpsimd.tensor_tensor(
                    out=u0[:], in0=X[:, 1 : S + 1], in1=M[:, 0:S], op=alu.subtract
                )
                st_a[i] = (i0, u0, cmx)

            def stage_u(i, split=False):
                i0, u0, cmx = st_a.pop(i)
                um = ump.tile([128, S], f32, tag="um")
                if split:
                    # last tile: halve the mask pass so Ln can start sooner;
                    # the two partial accums add up to the same ia.
                    Hh = S // 2
                    nc.vector.scalar_tensor_tensor(
                        out=um[:, 0:Hh], in0=cmx[:, 0:Hh],
                        scalar=float(C_THRESH), in1=u0[:, 0:Hh],
                        op0=alu.subtract, op1=alu.max,
                        accum_out=w2_sb[:, i : i + 1],
                    )
                    nc.vector.scalar_tensor_tensor(
                        out=um[:, Hh:S], in0=cmx[:, Hh:S],
                        scalar=float(C_THRESH), in1=u0[:, Hh:S],
                        op0=alu.subtract, op1=alu.max,
                        accum_out=acc_sb[:, i : i + 1],
                    )
                    nc.vector.tensor_tensor(
                        out=acc_sb[:, i : i + 1], in0=acc_sb[:, i : i + 1],
                        in1=w2_sb[:, i : i + 1], op=alu.add,
                    )
                else:
                    nc.vector.scalar_tensor_tensor(
                        out=um[:],
                        in0=cmx[:],
                        scalar=float(C_THRESH),
                        in1=u0[:],
                        op0=alu.subtract,
                        op1=alu.max,
                        accum_out=acc_sb[:, i : i + 1],
                    )
                st_u[i] = (i0, um)

            def stage_b(i, split=False):
                i0, um = st_u.pop(i)
                lnu = up.tile([128, S], f32, tag="lnu")
                r = up.tile([128, S], f32, tag="r")
                if split:
                    # last tile: halve the Ln/Exp/final chain to shrink the
                    # serial drain tail; partial row-sums add up afterwards.
                    Hh = S // 2
                    nc.scalar.activation(out=lnu[:, 0:Hh], in_=um[:, 0:Hh],
                                         func=act.Ln)
                    nc.scalar.activation(out=r[:, 0:Hh], in_=lnu[:, 0:Hh],
                                         func=act.Exp, scale=-1.0)
                    nc.scalar.activation(out=lnu[:, Hh:S], in_=um[:, Hh:S],
                                         func=act.Ln)
                    nc.scalar.activation(out=r[:, Hh:S], in_=lnu[:, Hh:S],
                                         func=act.Exp, scale=-1.0)
                    junk = ump.tile([128, S], f32, tag="um")
                    nc.vector.scalar_tensor_tensor(
                        out=junk[:, 0:Hh], in0=i0[:, 0:Hh], scalar=0.0,
                        in1=r[:, 0:Hh], op0=alu.max, op1=alu.mult,
                        accum_out=w_sb[:, i : i + 1],
                    )
                    nc.vector.scalar_tensor_tensor(
                        out=junk[:, Hh:S], in0=i0[:, Hh:S], scalar=0.0,
                        in1=r[:, Hh:S], op0=alu.max, op1=alu.mult,
                        accum_out=rs_sb[:, i : i + 1],
                    )
                    nc.vector.tensor_tensor(
                        out=rs_sb[:, i : i + 1], in0=rs_sb[:, i : i + 1],
                        in1=w_sb[:, i : i + 1], op=alu.add,
                    )
                else:
                    nc.scalar.activation(out=lnu[:], in_=um[:], func=act.Ln)
                    nc.scalar.activation(out=r[:], in_=lnu[:], func=act.Exp,
                                         scale=-1.0)
                    nc.vector.scalar_tensor_tensor(
                        out=um[:],
                        in0=i0[:],
                        scalar=0.0,
                        in1=r[:],
                        op0=alu.max,
                        op1=alu.mult,
                        accum_out=rs_sb[:, i : i + 1],
                    )

            for i in range(TILES):
                stage_a(i)
                if i >= UM_SKEW:
                    stage_u(i - UM_SKEW)
                if i >= B_SKEW:
                    stage_b(i - B_SKEW)
            for i in range(TILES - UM_SKEW, TILES):
                stage_u(i, split=(i == TILES - 1))
            # epilogue: num_seg = (S + MAGIC - acc/V) - MAGIC; out = rs/num_seg
            # Split by columns: tiles 0..6 finalize while tile 7 drains.
            def epilogue(lo, hi):
                nc.vector.scalar_tensor_tensor(
                    out=w_sb[:, lo:hi], in0=acc_sb[:, lo:hi],
                    scalar=-1.0 / V_INVALID, in1=carr[:, lo:hi],
                    op0=alu.mult, op1=alu.add,
                )
                nc.vector.tensor_scalar(
                    out=carr[:, lo:hi], in0=w_sb[:, lo:hi], scalar1=MAGIC,
                    scalar2=None, op0=alu.subtract,
                )
                nc.vector.reciprocal(out=w_sb[:, lo:hi], in_=carr[:, lo:hi])
                nc.vector.tensor_tensor(
                    out=carr[:, lo:hi], in0=rs_sb[:, lo:hi],
                    in1=w_sb[:, lo:hi], op=alu.mult
                )
                nc.sync.dma_start(out=o_d[:, lo:hi], in_=carr[:, lo:hi])

            done = False
            for i in range(TILES - B_SKEW, TILES):
                stage_b(i, split=(i == TILES - 1))
                if not done:
                    epilogue(0, TILES - 1)  # cols 0..6 ready after stage_b(6)
                    done = True
            epilogue(TILES - 1, TILES)
    return _legalize_waits(nc)


def _ensure_axon_visible():
    """If the caller pinned JAX_PLATFORMS=cpu (common in bench harnesses to
    keep the reference off-device) and jax is not yet initialized, lift the
    pin so the axon TRN2 backend this kernel executes on stays visible."""
    import os
    import sys

    plat = os.environ.get("JAX_PLATFORMS", "")
    if plat and "axon" not in plat and "jax" not in sys.modules:
        os.environ.pop("JAX_PLATFORMS", None)


def kernel(predictions: np.ndarray, targets: np.ndarray) -> np.ndarray:
    global _NC_CACHE
    _ensure_axon_visible()
    from concourse.bass_utils import run_bass_kernel_spmd

    if _NC_CACHE is None:
        _NC_CACHE = _build_nc()
    nc = _NC_CACHE

    p = np.ascontiguousarray(predictions, dtype=np.float32)
    t = np.ascontiguousarray(targets, dtype=np.float32)
    in_maps = []
    for c in range(NCORES):
        sl = slice(c * ROWS_PER_CORE, (c + 1) * ROWS_PER_CORE)
        in_maps.append({"predictions": p[sl], "targets": t[sl]})
    res = run_bass_kernel_spmd(nc, in_maps, core_ids=list(range(NCORES)))
    total = 0.0
    for rmap in res.results:
        total += float(rmap["row_iou"].astype(np.float64).sum())
    return np.asarray(1.0 - total / B, dtype=np.float32)



# revision 2
# speedup vs baseline: 1.5070x; 1.5070x over previous
"""DepthIoULoss kernel for Trainium2 (Bass/Tile), data-parallel over 8 cores.

Math (per row, S segments; v[-1] treated as 0): with M = min(p, t) and
X = max(p, t) elementwise:
    inter_j = relu(M_j - X_{j-1});  union_j = X_j - M_{j-1};  iou = inter/union
Valid prefix: j <= stop_idx, where stop_idx = first index with t == 1.0.
row_iou = sum_valid iou_j / (stop_idx + 1);  loss = 1 - mean_rows(row_iou).

Ragged trimming: each row only contributes lanes j <= stop_idx, and stop_idx
is uniform in [1, S-1].  kernel() sorts rows by stop position (descending),
deals them round-robin across the 8 cores (so every core sees the same
length profile and one SPMD module serves all), and trims tile k's loads and
every elementwise pass to L_k = (max stop in tile k) + 1 columns.  That cuts
both HBM traffic and engine work to ~56% of the dense schedule.  Sorting is
a sharding choice; all per-element math stays on device.  Rows without a
stop token contribute 0 (reference semantics), enforced via a host-side
has_stop mask on the returned per-row IoUs.

Device plan per [128, L] row-tile (only ops this walrus build accepts):
  ACT    tq  = t * K                      (K = 1e9, Copy w/ scale)
  DVE    cmx = exclusive-cummax(tq)       (tensor_tensor_scan, shifted AP)
  DVE    M   = min(p, t)                  ([128, L+1] tiles, zero column 0)
  DVE    X   = max(p, t) on cols [0, c)   (c ~ 0.75 L: DVE/Pool balance)
  GPSIMD X   = (p + t) - M on cols [c, L) (derived max; 1-ulp identical)
  GPSIMD i0  = M[:,1:] - X[:,:-1]
  GPSIMD u0  = X[:,1:] - M[:,:-1]
  DVE    um  = max(cmx - 0.95K, u0), accum -> ia   (invalid lanes -> V=0.05K;
                                                    ia = n_invalid*V + O(1e3))
  ACT    lnu = Ln(um);  r = Exp(-lnu) = 1/um       (invalid -> 2e-8)
  DVE    junk= max(i0,0) * r, accum -> rowsum      (relu fused here)
Epilogue on [128, 8]: num_seg = L_k - round(ia / V) (2^23 magic rounding),
row_iou = rowsum / num_seg -> DMA out. Host: loss = 1 - sum(row_iou) / B.

Manual software pipelining: engine queues run in EMISSION order, so the
um pass is emitted one tile late and the Ln/Exp/final passes two tiles
late, hiding Pool and ACT latency behind the next tile's DVE work.  Tiles
run longest-first, so the pipeline drains on the cheapest tiles.

The masked lanes contribute |inter|/V <= 2048 * 2e-8 ~ 4e-5 absolute to a
rowsum of O(1..30): negligible. num_seg recovery is exact (error << 0.5).
"""

import numpy as np

B, S = 8192, 2048
NCORES = 8
ROWS_PER_CORE = B // NCORES  # 1024
TILES = ROWS_PER_CORE // 128  # 8
STOP_TOKEN = np.float32(1.0)

K_SCALE = np.float32(1.0e9)  # ACT Ln accurate to ~1e16; keep um moderate
C_THRESH = np.float32(0.95) * K_SCALE
V_INVALID = float(np.float32(K_SCALE - np.float32(C_THRESH)))  # invalid-lane um
MAGIC = 8388608.0  # 2**23: float add/sub rounds to nearest integer

UM_SKEW = 1  # um pass trails stage A by one tile
B_SKEW = 2  # ln/exp/final trail stage A by two tiles

_NC_CACHE = None  # most recently built module (test.py reads this)
_NC_BY_PLAN = {}  # tile-length plan -> compiled module

_RANGE_CLEAR_OPCODE = 176  # EVENT_SEMAPHORE_RANGE_CLEAR


def _legalize_waits(nc, maxw=1):
    """Make the Tile-generated module compatible with this walrus build.

    1. Drop tail EVENT_SEMAPHORE_RANGE_CLEAR InstISA ops (NRT re-initializes
       semaphore state per execution; this walrus rejects the encoding).
    2. Split instructions carrying more than `maxw` sync waits: excess waits
       move to carrier EventSemaphore nops inserted just before, same engine.
    """
    import concourse.mybir as mybir

    uid = [0]
    for fn in nc.m.functions:
        for blk in fn.blocks:
            lst = blk.instructions
            k = 0
            while k < len(lst):
                inst = lst[k]
                if (
                    type(inst).__name__ == "InstISA"
                    and getattr(inst, "isa_opcode", None) == _RANGE_CLEAR_OPCODE
                ):
                    si = inst.sync_info
                    if si is not None and (si.on_wait or si.on_update):
                        carrier = mybir.InstEventSemaphore(name=f"RCW-{uid[0]}")
                        uid[0] += 1
                        carrier.engine = inst.engine
                        carrier.sync_info = si
                        lst[k] = carrier
                        k += 1
                    else:
                        del lst[k]
                    continue
                si = inst.sync_info
                if si is not None and si.on_wait and len(si.on_wait) > maxw:
                    waits = list(si.on_wait)
                    extra, keep = waits[:-maxw], waits[-maxw:]
                    pos = k
                    for j in range(0, len(extra), maxw):
                        carrier = mybir.InstEventSemaphore(name=f"EVW-{uid[0]}")
                        uid[0] += 1
                        carrier.engine = inst.engine
                        carrier.sync_info = mybir.SyncInfo(
                            on_wait=extra[j : j + maxw], on_update=[]
                        )
                        lst.insert(pos, carrier)
                        pos += 1
                        k += 1
                    inst.sync_info = mybir.SyncInfo(
                        on_wait=keep, on_update=list(si.on_update)
                    )
                k += 1
    return nc


def _build_nc(lens):
    """Build the 8-tile module for per-tile column lengths `lens` (desc)."""
    import concourse.bass as bass
    import concourse.mybir as mybir
    from concourse.tile import TileContext

    f32 = mybir.dt.float32
    alu = mybir.AluOpType
    act = mybir.ActivationFunctionType

    nc = bass.Bass()
    p_d = nc.dram_tensor("predictions", [ROWS_PER_CORE, S], f32, kind="ExternalInput")
    t_d = nc.dram_tensor("targets", [ROWS_PER_CORE, S], f32, kind="ExternalInput")
    o_d = nc.dram_tensor("row_iou", [128, TILES], f32, kind="ExternalOutput")

    with TileContext(nc) as tc:
        with (
            tc.tile_pool(name="io", bufs=2) as iop,
            tc.tile_pool(name="geom", bufs=2) as gp,
            tc.tile_pool(name="cmxp", bufs=3) as cp,
            tc.tile_pool(name="i0p", bufs=2) as i0p,
            tc.tile_pool(name="u0p", bufs=2) as u0p,
            tc.tile_pool(name="ump", bufs=3) as ump,
            tc.tile_pool(name="uch", bufs=2) as up,
            tc.tile_pool(name="sp", bufs=2) as spp,
            tc.tile_pool(name="smp", bufs=1) as smp,
        ):
            acc_sb = smp.tile([128, TILES], f32, tag="acc")
            rs_sb = smp.tile([128, TILES], f32, tag="rs")
            carr = smp.tile([128, TILES], f32, tag="carr")
            # per-tile num_seg base: L_k + MAGIC in column k
            for k in range(TILES):
                nc.vector.memset(carr[:, k : k + 1], float(lens[k]) + MAGIC)
            w_sb = smp.tile([128, TILES], f32, tag="w")

            st_a = {}
            st_u = {}

            def stage_a(i):
                L = lens[i]
                rows = slice(i * 128, (i + 1) * 128)
                # DVE/Pool balance: Pool derives max on the last ~25% of
                # columns via (p+t)-M, plus both shifted subtractions.
                c = min(L, max(1, (3 * L) // 4 + 76))
                p = iop.tile([128, L], f32, tag="p")
                t = iop.tile([128, L], f32, tag="t")
                tq = gp.tile([128, L], f32, tag="tq")
                cmx = cp.tile([128, L], f32, tag="cmx")
                M = gp.tile([128, L + 1], f32, tag="M")
                X = gp.tile([128, L + 1], f32, tag="X")
                nc.scalar.memzero(cmx[:, 0:1])
                nc.scalar.memzero(M[:, 0:1])
                nc.gpsimd.memset(X[:, 0:1], 0.0)
                # tile 0 is processed in column quarters so DVE ramps up while
                # the rest of the data is still in flight (chained scan).
                nparts = 4 if (i == 0 and L >= 64) else 1
                cuts = [round(L * q / nparts) for q in range(nparts + 1)]
                for k in range(nparts):
                    a, b = cuts[k], cuts[k + 1]
                    nc.sync.dma_start(out=t[:, a:b], in_=t_d[rows, a:b])
                    nc.sync.dma_start(out=p[:, a:b], in_=p_d[rows, a:b])
                    nc.scalar.activation(
                        out=tq[:, a:b], in_=t[:, a:b], func=act.Copy,
                        scale=float(K_SCALE),
                    )
                    # min first: Pool's derived-max part waits on M
                    nc.vector.tensor_tensor(
                        out=M[:, a + 1 : b + 1], in0=p[:, a:b], in1=t[:, a:b],
                        op=alu.min,
                    )
                    last = k == nparts - 1
                    if not last or b - a >= 2:
                        nc.vector.tensor_tensor_scan(
                            out=cmx[:, a + 1 : (b if last else b + 1)],
                            data0=tq[:, a : (b - 1 if last else b)],
                            data1=tq[:, a : (b - 1 if last else b)],
                            initial=(0.0 if k == 0 else cmx[:, a : a + 1]),
                            op0=alu.max,
                            op1=alu.bypass,
                        )
                    hi = min(b, c)
                    if hi > a:
                        nc.vector.tensor_tensor(
                            out=X[:, a + 1 : hi + 1], in0=p[:, a:hi],
                            in1=t[:, a:hi], op=alu.max,
                        )
                # Pool derives the remaining max columns: X = (p+t) - M
                if c < L:
                    sv = spp.tile([128, L - c], f32, tag="s")
                    nc.gpsimd.tensor_tensor(
                        out=sv[:], in0=p[:, c:L], in1=t[:, c:L], op=alu.add,
                    )
                    nc.gpsimd.tensor_tensor(
                        out=X[:, c + 1 : L + 1], in0=sv[:],
                        in1=M[:, c + 1 : L + 1], op=alu.subtract,
                    )
                i0 = i0p.tile([128, L], f32, tag="i0")
                nc.gpsimd.tensor_tensor(
                    out=i0[:], in0=M[:, 1 : L + 1], in1=X[:, 0:L], op=alu.subtract
                )
                u0 = u0p.tile([128, L], f32, tag="u0")
                nc.gpsimd.tensor_tensor(
                    out=u0[:], in0=X[:, 1 : L + 1], in1=M[:, 0:L], op=alu.subtract
                )
                st_a[i] = (i0, u0, cmx)

            def stage_u(i):
                L = lens[i]
                i0, u0, cmx = st_a.pop(i)
                um = ump.tile([128, L], f32, tag="um")
                nc.vector.scalar_tensor_tensor(
                    out=um[:],
                    in0=cmx[:],
                    scalar=float(C_THRESH),
                    in1=u0[:],
                    op0=alu.subtract,
                    op1=alu.max,
                    accum_out=acc_sb[:, i : i + 1],
                )
                st_u[i] = (i0, um)

            def stage_b(i):
                L = lens[i]
                i0, um = st_u.pop(i)
                lnu = up.tile([128, L], f32, tag="lnu")
                r = up.tile([128, L], f32, tag="r")
                nc.scalar.activation(out=lnu[:], in_=um[:], func=act.Ln)
                nc.scalar.activation(out=r[:], in_=lnu[:], func=act.Exp,
                                     scale=-1.0)
                nc.vector.scalar_tensor_tensor(
                    out=um[:],
                    in0=i0[:],
                    scalar=0.0,
                    in1=r[:],
                    op0=alu.max,
                    op1=alu.mult,
                    accum_out=rs_sb[:, i : i + 1],
                )

            for i in range(TILES):
                stage_a(i)
                if i >= UM_SKEW:
                    stage_u(i - UM_SKEW)
                if i >= B_SKEW:
                    stage_b(i - B_SKEW)
            for i in range(TILES - UM_SKEW, TILES):
                stage_u(i)
            done = False
            for i in range(TILES - B_SKEW, TILES):
                stage_b(i)
                if not done:
                    # cols 0..5 are final once stage_b(5) ran; finalize them
                    # while the two cheap tail tiles drain.
                    epilogue(nc, acc_sb, rs_sb, carr, w_sb, o_d, 0, TILES - B_SKEW)
                    done = True
            epilogue(nc, acc_sb, rs_sb, carr, w_sb, o_d, TILES - B_SKEW, TILES)
    return _legalize_waits(nc)


def epilogue(nc, acc_sb, rs_sb, carr, w_sb, o_d, lo, hi):
    """num_seg = (L_k + MAGIC - acc/V) - MAGIC; out = rs/num_seg."""
    import concourse.mybir as mybir

    alu = mybir.AluOpType
    nc.vector.scalar_tensor_tensor(
        out=w_sb[:, lo:hi], in0=acc_sb[:, lo:hi],
        scalar=-1.0 / V_INVALID, in1=carr[:, lo:hi],
        op0=alu.mult, op1=alu.add,
    )
    nc.vector.tensor_scalar(
        out=carr[:, lo:hi], in0=w_sb[:, lo:hi], scalar1=MAGIC,
        scalar2=None, op0=alu.subtract,
    )
    nc.vector.reciprocal(out=w_sb[:, lo:hi], in_=carr[:, lo:hi])
    nc.vector.tensor_tensor(
        out=carr[:, lo:hi], in0=rs_sb[:, lo:hi],
        in1=w_sb[:, lo:hi], op=alu.mult
    )
    nc.sync.dma_start(out=o_d[:, lo:hi], in_=carr[:, lo:hi])


def _ensure_axon_visible():
    """If the caller pinned JAX_PLATFORMS=cpu (common in bench harnesses to
    keep the reference off-device) and jax is not yet initialized, lift the
    pin so the axon TRN2 backend this kernel executes on stays visible."""
    import os
    import sys

    plat = os.environ.get("JAX_PLATFORMS", "")
    if plat and "axon" not in plat and "jax" not in sys.modules:
        os.environ.pop("JAX_PLATFORMS", None)


def kernel(predictions: np.ndarray, targets: np.ndarray) -> np.ndarray:
    global _NC_CACHE
    _ensure_axon_visible()
    from concourse.bass_utils import run_bass_kernel_spmd

    p = np.ascontiguousarray(predictions, dtype=np.float32)
    t = np.ascontiguousarray(targets, dtype=np.float32)

    # Row layout: sort by stop position (descending), deal round-robin across
    # cores.  Tile k of every core then spans the same global rank range, so
    # one module (with per-tile lengths) serves all 8 cores.
    stop_mask = t == STOP_TOKEN
    has_stop = stop_mask.any(axis=1)
    stops = np.argmax(stop_mask, axis=1).astype(np.int64)
    order = np.argsort(-stops, kind="stable")
    lens = tuple(
        int(min(S, stops[order[k * ROWS_PER_CORE]] + 1)) for k in range(TILES)
    )

    nc = _NC_BY_PLAN.get(lens)
    if nc is None:
        nc = _build_nc(lens)
        _NC_BY_PLAN[lens] = nc
    _NC_CACHE = nc

    in_maps = []
    core_rows = []
    for c in range(NCORES):
        rows = order[c::NCORES]
        core_rows.append(rows)
        in_maps.append({"predictions": p[rows], "targets": t[rows]})
    res = run_bass_kernel_spmd(nc, in_maps, core_ids=list(range(NCORES)))

    total = 0.0
    for c, rmap in enumerate(res.results):
        iou = rmap["row_iou"].astype(np.float64)  # [128, TILES]
        hs = has_stop[core_rows[c]].reshape(TILES, 128).T  # [128, TILES]
        total += float((iou * hs).sum())
    return np.asarray(1.0 - total / B, dtype=np.float32)
